# revision 1
# baseline (speedup 1.0000x reference)
"""nn_BlockPositioning: out[b*8+h, i, j] = ev_h[i//4, j//4] + c_h[i%4, j%4]

with ev_h[a, b] = eb_h[a-b] if a>b else ebf_h[b-a]  (Toeplitz in a-b); the
batch axis is a pure tile of the per-head bias.  Sharding: one head per core
(8 heads, 8 cores); the 4 identical batch copies are materialized host-side
at gather time.

Per-core device program (pure data movement + one fp32 add per unique value):
  Grev[s] = g_h[E-1-s]          host layout prep: reverse+concat, then per
                                partition p pre-shifted by p//4 zeros
                                (grev_shift[p, s] = Grev[s - p//4])
  S[p, 4s+jr] = grev_shift[p, s] + c_h[p%4, jr]    4x tensor_scalar_add (DVE)
    => S[p, x] = GI_{p%4}[x - 4*(p//4)],  GI_r[4s+jr] = Grev[s] + c[r, jr]
  out[128t+p, j] = S[p, (2044-128t)+j]             ONE 16 MiB DMA

The host-side pre-shift makes the output window start (2044-128t) identical
across partitions, so the bulk store is a single DMA of 2048 contiguous
8 KiB descriptors with a 128-way outer partition dim - it spreads over all
16 SDMA engines and runs at the HBM write roofline (~16 MiB/core).
"""

import numpy as np

_H = 8
_B = 4
_E = 512
_SEQ = 4 * _E              # 2048
_GLEN = 2 * _E - 1         # 1023
_NT = _SEQ // 128          # 16
_SLEN = _GLEN + 31         # 1054: shifted grev row length
_SROW = 4 * _SLEN          # 4216: S row length
_X0 = 4 * (_E - 1)         # 2044: window start for t=0

_CACHE = {}


def _build_nc():
    import concourse.bass as bass
    import concourse.mybir as mybir

    F32 = mybir.dt.float32
    nc = bass.Bass()
    grev_in = nc.dram_tensor("grev", [128, _SLEN], F32, kind="ExternalInput")
    cmat_in = nc.dram_tensor("cmat", [128, 4], F32, kind="ExternalInput")
    out = nc.dram_tensor("out", [_SEQ, _SEQ], F32, kind="ExternalOutput")

    with (
        nc.sbuf_tensor([128, _SLEN], F32) as grev_sb,
        nc.sbuf_tensor([128, 4], F32) as c_sb,
        nc.sbuf_tensor([128, _SROW], F32) as s2,
        nc.semaphore("dma_sem") as dma_sem,
        nc.semaphore("v_sem") as v_sem,
        nc.Block() as block,
    ):
        # S[p, 4s+jr] = grev_shift[p, s] + c[p%4, jr]   (strided dest view)
        s3 = s2[:, :].rearrange("p (s j) -> p s j", j=4)

        # phase A covers s in [511, 1054) - everything the t=0 window reads -
        # so the bulk store can start after ~1/2 of the add work; phase B
        # fills s in [0, 511) for the remaining 15 window blocks.
        _SA = 511

        @block.vector
        def _(vector):
            vector.wait_ge(dma_sem, 32)  # grev + cmat resident
            for jr in range(4):
                vector.tensor_scalar_add(
                    out=s3[:, _SA:, jr],
                    in0=grev_sb[:, _SA:],
                    scalar1=c_sb[:, jr : jr + 1],
                ).then_inc(v_sem, 1)
            for jr in range(4):
                vector.tensor_scalar_add(
                    out=s3[:, :_SA, jr],
                    in0=grev_sb[:, :_SA],
                    scalar1=c_sb[:, jr : jr + 1],
                ).then_inc(v_sem, 1)

        @block.sync
        def _(sync):
            sync.dma_start(out=grev_sb[:, :], in_=grev_in[:, :]).then_inc(dma_sem, 16)
            sync.dma_start(out=c_sb[:, :], in_=cmat_in[:, :]).then_inc(dma_sem, 16)
            sb = s2[:, :]
            # out[128t + p, j] = S[p, (2044 - 128t) + j]
            sync.wait_ge(v_sem, 4)  # phase A ready: rows 0..127 (t=0)
            src_a = bass.AP(sb.tensor, sb.offset + _X0, [[_SROW, 128], [1, _SEQ]])
            with nc.allow_non_contiguous_dma(reason="toeplitz windows"):
                sync.dma_start(out=out[0:128, :], in_=src_a).then_inc(dma_sem, 16)
            sync.wait_ge(v_sem, 8)  # phase B ready: rows 128..2047
            # one DMA per t-block: dest rows [128t, 128t+128) sweep DRAM
            # linearly (8 KiB writes at consecutive addresses), keeping the
            # 128-way outer dim that spreads over all 16 SDMA engines.
            for t in range(1, _NT):
                src_b = bass.AP(
                    sb.tensor,
                    sb.offset + _X0 - 128 * t,
                    [[_SROW, 128], [1, _SEQ]],
                )
                with nc.allow_non_contiguous_dma(reason="toeplitz windows"):
                    sync.dma_start(
                        out=out[128 * t : 128 * (t + 1), :], in_=src_b
                    ).then_inc(dma_sem, 16)
            sync.wait_ge(dma_sem, 16 * (3 + _NT - 1))

    return nc


def _in_maps(channel_blocks, event_blocks, event_blocks_future):
    maps = []
    for h in range(_H):
        eb = np.ascontiguousarray(event_blocks[:, 0, h], dtype=np.float32)
        ebf = np.ascontiguousarray(event_blocks_future[:, 0, h], dtype=np.float32)
        grev = np.concatenate([eb[_E - 1 : 0 : -1], ebf])  # (1023,)
        # row p: p//4 leading zeros, grev, zeros to length SLEN
        gs = np.zeros((128, _SLEN), dtype=np.float32)
        for q in range(32):
            gs[4 * q : 4 * q + 4, q : q + _GLEN] = grev
        c = np.ascontiguousarray(channel_blocks[:, :, 0, h], dtype=np.float32)  # (4,4)
        maps.append(
            {
                "grev": gs,
                "cmat": np.ascontiguousarray(np.tile(c, (32, 1)), dtype=np.float32),
            }
        )
    return maps


def _compiled_runner():
    """Build (once) a jitted 8-core runner mirroring bass2jax.run_bass_via_pjrt,
    so repeat kernel() calls reuse the compiled NEFF executable."""
    if "runner" in _CACHE:
        return _CACHE["runner"]

    import jax
    import concourse.mybir as mybir
    from concourse import bass2jax
    from jax.experimental.shard_map import shard_map
    from jax.sharding import Mesh, PartitionSpec

    bass2jax.install_neuronx_cc_hook()
    if "nc" not in _CACHE:
        _CACHE["nc"] = _build_nc()
    nc = _CACHE["nc"]

    partition_name = nc.partition_id_tensor.name if nc.partition_id_tensor else None
    in_names, out_names, out_avals, zero_outs = [], [], [], []
    for alloc in nc.m.functions[0].allocations:
        if not isinstance(alloc, mybir.MemoryLocationSet):
            continue
        name = alloc.memorylocations[0].name
        if alloc.kind == "ExternalInput":
            if name != partition_name:
                in_names.append(name)
        elif alloc.kind == "ExternalOutput":
            shape = tuple(alloc.tensor_shape)
            dtype = mybir.dt.np(alloc.dtype)
            out_names.append(name)
            out_avals.append(jax.core.ShapedArray(shape, dtype))
            zero_outs.append(np.zeros(shape, dtype))
    n_params = len(in_names)
    all_in_names = in_names + out_names
    if partition_name is not None:
        all_in_names = all_in_names + [partition_name]
    all_in_names = tuple(all_in_names)

    def _body(*args):
        operands = list(args)
        if partition_name is not None:
            operands.append(bass2jax.partition_id_tensor())
        return tuple(
            bass2jax._bass_exec_p.bind(
                *operands,
                out_avals=tuple(out_avals),
                in_names=all_in_names,
                out_names=tuple(out_names),
                lowering_input_output_aliases=(),
                sim_require_finite=True,
                sim_require_nnan=True,
                nc=nc,
            )
        )

    devices = jax.devices()[:_H]
    mesh = Mesh(np.asarray(devices), ("core",))
    donate = tuple(range(n_params, n_params + len(out_names)))
    sharded = jax.jit(
        shard_map(
            _body,
            mesh=mesh,
            in_specs=(PartitionSpec("core"),) * (n_params + len(out_names)),
            out_specs=(PartitionSpec("core"),) * len(out_names),
            check_rep=False,
        ),
        donate_argnums=donate,
        keep_unused=True,
    )

    def run(in_maps):
        concat_in = [
            np.concatenate([m[name] for m in in_maps], axis=0) for name in in_names
        ]
        concat_zeros = [
            np.zeros((_H * z.shape[0], *z.shape[1:]), z.dtype) for z in zero_outs
        ]
        out_arrs = sharded(*concat_in, *concat_zeros)
        return [
            {
                name: np.asarray(out_arrs[i]).reshape(_H, *out_avals[i].shape)[c]
                for i, name in enumerate(out_names)
            }
            for c in range(_H)
        ]

    _CACHE["runner"] = run
    return run


def run_spmd(channel_blocks, event_blocks, event_blocks_future):
    """Run the per-head kernels on cores 0-7; returns (None, heads).

    heads: float32 (8, 2048, 2048), one bias matrix per head."""
    run = _compiled_runner()
    results = run(_in_maps(channel_blocks, event_blocks, event_blocks_future))
    heads = np.stack([np.asarray(results[h]["out"]) for h in range(_H)])
    return None, heads


def kernel(q, channel_blocks, event_blocks, event_blocks_future):
    q = np.asarray(q)
    channel_blocks = np.asarray(channel_blocks, dtype=np.float32)
    event_blocks = np.asarray(event_blocks, dtype=np.float32)
    event_blocks_future = np.asarray(event_blocks_future, dtype=np.float32)

    _, heads = run_spmd(channel_blocks, event_blocks, event_blocks_future)
    batch = q.shape[0] // _H
    return np.tile(heads, (batch, 1, 1))



# revision 2
# speedup vs baseline: 1.0251x; 1.0251x over previous
"""nn_BlockPositioning: out[b*8+h, i, j] = ev_h[i//4, j//4] + c_h[i%4, j%4]

with ev_h[a, b] = eb_h[a-b] if a>b else ebf_h[b-a]  (Toeplitz in a-b); the
batch axis is a pure tile of the per-head bias.  Sharding: one head per core
(8 heads, 8 cores); the 4 identical batch copies are materialized host-side
at gather time.

Per-core device program (pure data movement + one fp32 add per unique value):
  Grev[s] = g_h[E-1-s]          host layout prep: reverse+concat, then per
                                partition p pre-shifted by p//4 zeros
                                (grev_shift[p, s] = Grev[s - p//4]); cmat
                                rows appended as cols [1054, 1058)
  S[p, 4s+jr] = grev_shift[p, s] + c_h[p%4, jr]    tensor_scalar_add (DVE)
    => S[p, x] = GI_{p%4}[x - 4*(p//4)],  GI_r[4s+jr] = Grev[s] + c[r, jr]
  out[128t+p, j] = S[p, (2044-128t)+j]             3 merged window DMAs

The host-side pre-shift makes the output window start (2044-128t) identical
across partitions, so the bulk store is 3 DMAs (t=0 / t=1-4 / t=5-15) whose
t-level is folded into the access pattern with a negative source stride.
Merging the per-window DMAs into 3 instructions removes the per-dma_start
issue gaps and cuts semaphore-update traffic; splitting the input load and
the adds into 3 phases (s>=511 / [383,511) / rest) lets the first window's
store start as early as possible while later phases hide under it.  The 8KiB
descriptors spread over all 16 SDMA engines and run at the HBM write
roofline (~16 MiB/core).
"""

import numpy as np

_H = 8
_B = 4
_E = 512
_SEQ = 4 * _E              # 2048
_GLEN = 2 * _E - 1         # 1023
_NT = _SEQ // 128          # 16
_SLEN = _GLEN + 31         # 1054: shifted grev row length
_GW = _SLEN + 4            # 1058: grev row + 4 cmat columns
_SROW = 4 * _SLEN          # 4216: S row length
_X0 = 4 * (_E - 1)         # 2044: window start for t=0
_SA = 511                  # phase A adds cover s in [511, 1054)
_SB1 = 383                 # phase B1 adds cover s in [383, 511)

_CACHE = {}


def _build_nc():
    import concourse.bass as bass
    import concourse.mybir as mybir

    F32 = mybir.dt.float32
    nc = bass.Bass()
    gin = nc.dram_tensor("gin", [128, _GW], F32, kind="ExternalInput")
    out = nc.dram_tensor("out", [_SEQ, _SEQ], F32, kind="ExternalOutput")

    with (
        nc.sbuf_tensor([128, _GW], F32) as gbuf,
        nc.sbuf_tensor([128, _SROW], F32) as s2,
        nc.semaphore("hi_sem") as hi_sem,
        nc.semaphore("dma_sem") as dma_sem,
        nc.semaphore("v_sem") as v_sem,
        nc.Block() as block,
    ):
        # S[p, 4s+jr] = grev_shift[p, s] + c[p%4, jr]   (strided dest view)
        s3 = s2[:, :].rearrange("p (s j) -> p s j", j=4)

        @block.vector
        def _(vector):
            # phase A: everything the t=0 window reads
            vector.wait_ge(hi_sem, 16)
            for jr in range(4):
                vector.tensor_scalar_add(
                    out=s3[:, _SA:, jr],
                    in0=gbuf[:, _SA:_SLEN],
                    scalar1=gbuf[:, _SLEN + jr : _SLEN + jr + 1],
                ).then_inc(v_sem, 1)
            # phase B1: unlocks windows t=1..4
            vector.wait_ge(dma_sem, 16)
            for jr in range(4):
                vector.tensor_scalar_add(
                    out=s3[:, _SB1:_SA, jr],
                    in0=gbuf[:, _SB1:_SA],
                    scalar1=gbuf[:, _SLEN + jr : _SLEN + jr + 1],
                ).then_inc(v_sem, 1)
            # phase B2: remaining columns for windows t=5..15
            for jr in range(4):
                vector.tensor_scalar_add(
                    out=s3[:, :_SB1, jr],
                    in0=gbuf[:, :_SB1],
                    scalar1=gbuf[:, _SLEN + jr : _SLEN + jr + 1],
                ).then_inc(v_sem, 1)

        @block.sync
        def _(sync):
            # split input load: high columns (phase A + cmat) land first
            sync.dma_start(out=gbuf[:, _SA:], in_=gin[:, _SA:]).then_inc(hi_sem, 16)
            sync.dma_start(out=gbuf[:, :_SA], in_=gin[:, :_SA]).then_inc(dma_sem, 16)
            sb = s2[:, :]
            ob = out[:, :]
            # out[128t + p, j] = S[p, (2044 - 128t) + j]; the t level is an
            # AP dimension (src stride -128, dst stride 128*2048)
            with nc.allow_non_contiguous_dma(reason="toeplitz windows"):
                sync.wait_ge(v_sem, 4)
                src = bass.AP(sb.tensor, sb.offset + _X0, [[_SROW, 128], [1, _SEQ]])
                sync.dma_start(out=out[0:128, :], in_=src).then_inc(dma_sem, 16)
                sync.wait_ge(v_sem, 8)
                src = bass.AP(
                    sb.tensor,
                    sb.offset + _X0 - 128,
                    [[_SROW, 128], [-128, 4], [1, _SEQ]],
                )
                dst = bass.AP(
                    ob.tensor,
                    ob.offset + 128 * _SEQ,
                    [[_SEQ, 128], [128 * _SEQ, 4], [1, _SEQ]],
                )
                sync.dma_start(out=dst, in_=src).then_inc(dma_sem, 16)
                sync.wait_ge(v_sem, 12)
                src = bass.AP(
                    sb.tensor,
                    sb.offset + _X0 - 128 * 5,
                    [[_SROW, 128], [-128, 11], [1, _SEQ]],
                )
                dst = bass.AP(
                    ob.tensor,
                    ob.offset + 5 * 128 * _SEQ,
                    [[_SEQ, 128], [128 * _SEQ, 11], [1, _SEQ]],
                )
                sync.dma_start(out=dst, in_=src).then_inc(dma_sem, 16)
            sync.wait_ge(dma_sem, 16 * 4)
            sync.wait_ge(hi_sem, 16)

    return nc


def _in_maps(channel_blocks, event_blocks, event_blocks_future):
    maps = []
    for h in range(_H):
        eb = np.ascontiguousarray(event_blocks[:, 0, h], dtype=np.float32)
        ebf = np.ascontiguousarray(event_blocks_future[:, 0, h], dtype=np.float32)
        grev = np.concatenate([eb[_E - 1 : 0 : -1], ebf])  # (1023,)
        # row p: p//4 leading zeros, grev, zeros to length SLEN; then 4 cmat cols
        gs = np.zeros((128, _GW), dtype=np.float32)
        for q in range(32):
            gs[4 * q : 4 * q + 4, q : q + _GLEN] = grev
        c = np.ascontiguousarray(channel_blocks[:, :, 0, h], dtype=np.float32)  # (4,4)
        gs[:, _SLEN:] = np.tile(c, (32, 1))
        maps.append({"gin": gs})
    return maps


def _compiled_runner():
    """Build (once) a jitted 8-core runner mirroring bass2jax.run_bass_via_pjrt,
    so repeat kernel() calls reuse the compiled NEFF executable."""
    if "runner" in _CACHE:
        return _CACHE["runner"]

    import jax
    import concourse.mybir as mybir
    from concourse import bass2jax
    from jax.experimental.shard_map import shard_map
    from jax.sharding import Mesh, PartitionSpec

    bass2jax.install_neuronx_cc_hook()
    if "nc" not in _CACHE:
        _CACHE["nc"] = _build_nc()
    nc = _CACHE["nc"]

    partition_name = nc.partition_id_tensor.name if nc.partition_id_tensor else None
    in_names, out_names, out_avals, zero_outs = [], [], [], []
    for alloc in nc.m.functions[0].allocations:
        if not isinstance(alloc, mybir.MemoryLocationSet):
            continue
        name = alloc.memorylocations[0].name
        if alloc.kind == "ExternalInput":
            if name != partition_name:
                in_names.append(name)
        elif alloc.kind == "ExternalOutput":
            shape = tuple(alloc.tensor_shape)
            dtype = mybir.dt.np(alloc.dtype)
            out_names.append(name)
            out_avals.append(jax.core.ShapedArray(shape, dtype))
            zero_outs.append(np.zeros(shape, dtype))
    n_params = len(in_names)
    all_in_names = in_names + out_names
    if partition_name is not None:
        all_in_names = all_in_names + [partition_name]
    all_in_names = tuple(all_in_names)

    def _body(*args):
        operands = list(args)
        if partition_name is not None:
            operands.append(bass2jax.partition_id_tensor())
        return tuple(
            bass2jax._bass_exec_p.bind(
                *operands,
                out_avals=tuple(out_avals),
                in_names=all_in_names,
                out_names=tuple(out_names),
                lowering_input_output_aliases=(),
                sim_require_finite=True,
                sim_require_nnan=True,
                nc=nc,
            )
        )

    devices = jax.devices()[:_H]
    mesh = Mesh(np.asarray(devices), ("core",))
    donate = tuple(range(n_params, n_params + len(out_names)))
    sharded = jax.jit(
        shard_map(
            _body,
            mesh=mesh,
            in_specs=(PartitionSpec("core"),) * (n_params + len(out_names)),
            out_specs=(PartitionSpec("core"),) * len(out_names),
            check_rep=False,
        ),
        donate_argnums=donate,
        keep_unused=True,
    )

    def run(in_maps):
        concat_in = [
            np.concatenate([m[name] for m in in_maps], axis=0) for name in in_names
        ]
        concat_zeros = [
            np.zeros((_H * z.shape[0], *z.shape[1:]), z.dtype) for z in zero_outs
        ]
        out_arrs = sharded(*concat_in, *concat_zeros)
        return [
            {
                name: np.asarray(out_arrs[i]).reshape(_H, *out_avals[i].shape)[c]
                for i, name in enumerate(out_names)
            }
            for c in range(_H)
        ]

    _CACHE["runner"] = run
    return run


def run_spmd(channel_blocks, event_blocks, event_blocks_future):
    """Run the per-head kernels on cores 0-7; returns (None, heads).

    heads: float32 (8, 2048, 2048), one bias matrix per head."""
    run = _compiled_runner()
    results = run(_in_maps(channel_blocks, event_blocks, event_blocks_future))
    heads = np.stack([np.asarray(results[h]["out"]) for h in range(_H)])
    return None, heads


def kernel(q, channel_blocks, event_blocks, event_blocks_future):
    q = np.asarray(q)
    channel_blocks = np.asarray(channel_blocks, dtype=np.float32)
    event_blocks = np.asarray(event_blocks, dtype=np.float32)
    event_blocks_future = np.asarray(event_blocks_future, dtype=np.float32)

    _, heads = run_spmd(channel_blocks, event_blocks, event_blocks_future)
    batch = q.shape[0] // _H
    return np.tile(heads, (batch, 1, 1))
